# revision 59
# baseline (speedup 1.0000x reference)
"""GraphTransformerLayer kernel for 8 Trainium2 NeuronCores.

Sharding: graphs sorted by size into 4 bands of 8; core c takes the c-th
graph of each band (slot). Slot k is padded to the band max (rounded to 4),
so all cores run one identical SPMD program with near-zero padding waste.
Cores are fully independent (no collectives).

Layout is feature-major [dim, node] for q/k/cT/out so matmuls need no
transposes; v is node-major per 128-row key block. Per-core validity is
pure data (zero-padded x, a 0/1 vmask multiplied into v), so the exp needs
no mask bias and fuses across key blocks when 2*S <= 512.

Numerics: b_k is dropped (softmax-invariant), b_v is folded into b_o
(softmax weights sum to 1). All matmul inputs bf16, fp32 PSUM; softmax
denominators via a ones-column in v; reciprocal broadcast to 64 rows with
a K=2 matmul per head pair.

Projections run as fp8(e4m3) hi+lo residual DoubleRow matmuls
(x@W ~ xh@Wh + xh@Wl + xl@Wh at 0.5 cycles/row, pre-scaled by 32/512 to
keep residuals out of the fp8 denormal range) which beats bf16 in both
speed and accuracy; the out-projection stays bf16 (its input is produced
on device). Engine balance: PE matmuls; ACT exp + rb/o-bias copies; DVE
q/k/v copies, reciprocals, normalize multiplies (GPSIMD cannot access
PSUM). Projection and out-projection chunks are interleaved into the
attention step stream with adaptive draining so the PE stays fed while
the ACT exp chain runs.
"""

import os
import sys
from collections import deque

import numpy as np
import ml_dtypes

for _p in ("/opt/trn_rl_repo", "/root/.axon_site/_ro/trn_rl_repo"):
    if os.path.isdir(_p) and _p not in sys.path:
        sys.path.insert(0, _p)

DIM = 512
H = 8
DH = 64
NUM_GRAPHS = 32
N_CORES = 8
NS = NUM_GRAPHS // N_CORES  # slots (graphs) per core
VC = H * (DH + 1)  # 520: v columns (per head: 64 dims + 1 ones col)
SCALE = 1.0 / np.sqrt(DH)
SX, SW = 32.0, 512.0  # fp8 pre-scales (power of two) for x and w_{q,k,v}
ISCALE = 1.0 / (SX * SW)

_NC_CACHE = {}
LAST_RESULTS = None



def _build(S):
    import concourse.bass as bass
    import concourse.tile as tile
    from concourse import mybir
    from contextlib import ExitStack

    f32 = mybir.dt.float32
    b16 = mybir.dt.bfloat16
    AF = mybir.ActivationFunctionType

    S = [int(s) for s in S]
    JT = [-(-s // 128) for s in S]
    JSPAN = [jt * 128 for jt in JT]
    O = np.concatenate([[0], np.cumsum(JSPAN)]).astype(int)  # key layout
    P = np.concatenate([[0], np.cumsum(S)]).astype(int)  # query layout
    NPX, NPAD = int(O[NS]), int(P[NS])
    TOTJB = sum(JT)
    jbcol = {}  # (g, jb) -> column in vmask
    for g in range(NS):
        for jb in range(JT[g]):
            jbcol[(g, jb)] = len(jbcol)
    # j-block grouping per slot for score psum tiles / fused exp
    groups = {}
    for g in range(NS):
        if S[g] * JT[g] <= 512:
            groups[g] = [list(range(JT[g]))]
        else:
            groups[g] = [[jb] for jb in range(JT[g])]

    f8 = mybir.dt.float8e4
    nc = bass.Bass()
    # x / w_{q,k,v} as fp8 hi+lo residual pairs in DoubleRow pair layout:
    # tensor[t][p, i, n] = src[256*t + 128*i + p, n] for kb-pair t, slice i
    x8_d = {}
    w8_d = {}
    for hl in ("h", "l"):
        for t in range(2):
            x8_d[(hl, t)] = nc.declare_dram_parameter(
                f"x{hl}{t}", [128, 2, NPX], f8, isOutput=False)
            for w in ("q", "k", "v"):
                cols = VC if w == "v" else DIM
                w8_d[(w, hl, t)] = nc.declare_dram_parameter(
                    f"w{w}{hl}{t}", [128, 2, cols], f8, isOutput=False)
    wo_d = nc.declare_dram_parameter("woT", [DIM, DIM], b16, isOutput=False)
    bq_d = nc.declare_dram_parameter("bq", [128, 4], f32, isOutput=False)
    bo_d = nc.declare_dram_parameter("bo2", [128, 4], f32, isOutput=False)
    v8_d = nc.declare_dram_parameter("vones", [128, 8 * TOTJB], b16, isOutput=False)
    out_d = nc.declare_dram_parameter("outT", [DIM, NPAD], b16, isOutput=True)

    with ExitStack() as ctx:
        tc = ctx.enter_context(tile.TileContext(nc))
        wpool = ctx.enter_context(tc.tile_pool(name="w", bufs=1))
        xpool = ctx.enter_context(tc.tile_pool(name="x", bufs=1))
        apool = ctx.enter_context(tc.tile_pool(name="acts", bufs=1))
        vpool = ctx.enter_context(tc.tile_pool(name="v", bufs=1))
        epool = ctx.enter_context(tc.tile_pool(name="e", bufs=12))
        rcpool = ctx.enter_context(tc.tile_pool(name="rc", bufs=6))
        rbpool = ctx.enter_context(tc.tile_pool(name="rb", bufs=4))
        opool = ctx.enter_context(tc.tile_pool(name="o", bufs=4))
        ps = ctx.enter_context(tc.tile_pool(name="ps", bufs=2, space="PSUM"))
        sc = ctx.enter_context(tc.tile_pool(name="sc", bufs=3, space="PSUM"))
        avp = ctx.enter_context(tc.tile_pool(name="avp", bufs=3, space="PSUM"))

        # ---- loads (ordered: first-needed first; k proj starts per-kb)
        # loads spread over three HWDGE queues (SP: x; ACT: wk/wq;
        # plus wv/masks on SP, wo on ACT) so the prologue isn't serialized on one queue
        x_sb, w_sb, wo_sb = {}, {}, []
        for t in range(2):
            for hl in ("h", "l"):
                tl = xpool.tile([128, 2, NPX], f8, tag=f"x{hl}{t}",
                                name=f"x{hl}{t}")
                nc.sync.dma_start(tl[:, :, :], x8_d[(hl, t)][:, :, :])
                x_sb[(hl, t)] = tl
                tl = wpool.tile([128, 2, DIM], f8, tag=f"wk{hl}{t}",
                                name=f"wk{hl}{t}")
                nc.scalar.dma_start(tl[:, :, :], w8_d[("k", hl, t)][:, :, :])
                w_sb[("k", hl, t)] = tl
        for t in range(2):
            for hl in ("h", "l"):
                tl = wpool.tile([128, 2, DIM], f8, tag=f"wq{hl}{t}",
                                name=f"wq{hl}{t}")
                nc.scalar.dma_start(tl[:, :, :], w8_d[("q", hl, t)][:, :, :])
                w_sb[("q", hl, t)] = tl
        for t in range(2):
            for hl in ("h", "l"):
                tl = wpool.tile([128, 2, VC], f8, tag=f"wv{hl}{t}",
                                name=f"wv{hl}{t}")
                nc.sync.dma_start(tl[:, :, :], w8_d[("v", hl, t)][:, :, :])
                w_sb[("v", hl, t)] = tl
        bq_sb = wpool.tile([128, 4], f32, tag="bq")
        nc.scalar.dma_start(bq_sb[:], bq_d[:])
        v8_sb = wpool.tile([128, 8 * TOTJB], b16, tag="v8")
        nc.sync.dma_start(v8_sb[:], v8_d[:])
        for kb in range(4):
            t = wpool.tile([128, DIM], b16, tag=f"wo{kb}", name=f"wo{kb}")
            nc.scalar.dma_start(t[:], wo_d[kb * 128:(kb + 1) * 128, :])
            wo_sb.append(t)
        bo_sb = wpool.tile([128, 4], f32, tag="bo")
        nc.scalar.dma_start(bo_sb[:], bo_d[:])

        ones1 = wpool.tile([1, DH], b16, tag="ones1")
        nc.gpsimd.memset(ones1[:], 1.0)

        # keep the PE continuously busy through the DMA prologue so the
        # p-state is ramped when the projection burst hits the critical path
        wu = wpool.tile([128, 512], b16, tag="wu")
        nc.gpsimd.memset(wu[:], 0.0)
        wup = ps.tile([128, 512], f32, tag="ps", name="wup")
        for _ in range(15):
            nc.tensor.matmul(wup[:], wu[:, :128], wu[:], start=True, stop=True)


        # persistent activations
        qT_sb = [apool.tile([128, NPAD], b16, tag=f"q{fb}", name=f"qT{fb}") for fb in range(4)]
        kT_sb = [apool.tile([128, NPX], b16, tag=f"k{fb}", name=f"kT{fb}") for fb in range(4)]
        for fb in range(4):
            for g in range(NS):
                if JSPAN[g] > S[g]:
                    nc.gpsimd.memset(kT_sb[fb][:, O[g] + S[g]:O[g] + JSPAN[g]],
                                     0.0)
        cT_sb = [apool.tile([128, NPAD], b16, tag=f"c{fb}", name=f"cT{fb}") for fb in range(4)]
        v_sb = {}
        for g in range(NS):
            for jb in range(JT[g]):
                v_sb[(g, jb)] = vpool.tile([128, VC], b16, tag=f"v{g}_{jb}",
                                           name=f"v{g}_{jb}")

        # ---- projection chunks: fp8 hi/lo residual DoubleRow matmuls.
        # x@W ~ xh@Wh + xh@Wl + xl@Wh, each pass 2 DR matmuls (K=256),
        # psum scaled back by 1/(SX*SW) during the copy out.
        DR = mybir.MatmulPerfMode.DoubleRow
        PASSES = (("h", "h"), ("h", "l"), ("l", "h"))

        def dr_proj(p, w, wcols, xcols, width):
            n = 0
            for (xhl, whl) in PASSES:
                for t in range(2):
                    n += 1
                    nc.tensor.matmul(
                        p[:, :width],
                        w_sb[(w, whl, t)][:, :, wcols],
                        x_sb[(xhl, t)][:, :, xcols],
                        start=(n == 1), stop=(n == 6),
                        perf_mode=DR,
                    )

        def k_chunk(g, fb, pool=None, ptag="ps"):
            def emit():
                pl = pool or ps
                span = S[g]
                p = pl.tile([128, 512], f32, tag=ptag, name="psk")
                dr_proj(p, "k", slice(fb * 128, (fb + 1) * 128),
                        slice(O[g], O[g] + span), span)
                nc.vector.tensor_scalar_mul(
                    kT_sb[fb][:, O[g]:O[g] + span], p[:, :span], ISCALE)
            return emit

        def q_chunk(g, fb, pool=None, ptag="ps"):
            def emit():
                pl = pool or ps
                s = S[g]
                p = pl.tile([128, 512], f32, tag=ptag, name="psq")
                dr_proj(p, "q", slice(fb * 128, (fb + 1) * 128),
                        slice(O[g], O[g] + s), s)
                nc.vector.tensor_scalar(
                    qT_sb[fb][:, P[g]:P[g] + s], p[:, :s], ISCALE,
                    bq_sb[:, fb:fb + 1], mybir.AluOpType.mult,
                    mybir.AluOpType.add)
            return emit

        def v_chunk(g, jb, pool=None, ptag="ps"):
            def emit():
                pl = pool or ps
                vt = v_sb[(g, jb)]
                col0 = O[g] + jb * 128
                mcol = jbcol[(g, jb)]
                for (off, w) in ((0, 512), (512, VC - 512)):
                    p = pl.tile([128, 512], f32, tag=ptag, name="psv")
                    n = 0
                    for (xhl, whl) in PASSES:
                        for t in range(2):
                            n += 1
                            nc.tensor.matmul(
                                p[:, :w],
                                x_sb[(xhl, t)][:, :, col0:col0 + 128],
                                w_sb[("v", whl, t)][:, :, off:off + w],
                                start=(n == 1), stop=(n == 6),
                                perf_mode=DR,
                            )
                    nc.vector.tensor_scalar_mul(vt[:, off:off + w], p[:, :w],
                                                ISCALE)
                # ones columns (validity) overwrite the zero wv columns
                nc.gpsimd.tensor_copy(
                    vt[:, DH::DH + 1], v8_sb[:, mcol * 8:(mcol + 1) * 8])
            return emit

        def o_chunk(g, fb):
            def emit():
                s = S[g]
                p = ps.tile([128, 512], f32, tag="ps", name="pso")
                for kb in range(4):
                    nc.tensor.matmul(
                        p[:, :s],
                        wo_sb[kb][:, fb * 128:(fb + 1) * 128],
                        cT_sb[kb][:, P[g]:P[g] + s],
                        start=(kb == 0), stop=(kb == 3),
                    )
                ot = opool.tile([128, 512], b16, tag="ot", name="ot")
                if g == NS - 1:
                    nc.vector.tensor_scalar_add(ot[:, :s], p[:, :s],
                                                bo_sb[:, fb:fb + 1])
                else:
                    nc.scalar.activation(ot[:, :s], p[:, :s], AF.Identity,
                                         bias=bo_sb[:, fb:fb + 1])
                dma_eng = nc.sync if fb % 2 == 0 else nc.scalar
                dma_eng.dma_start(
                    out_d[fb * 128:(fb + 1) * 128, P[g]:P[g] + s], ot[:, :s])
            return emit

        def proj_chunks(g, pool=None, ptag="ps"):
            out = []
            for fb in range(4):
                out.append(k_chunk(g, fb, pool, ptag))
            for fb in range(4):
                out.append(q_chunk(g, fb, pool, ptag))
            for jb in range(JT[g]):
                out.append(v_chunk(g, jb, pool, ptag))
            return out

        # ---- attention step pieces
        def emit_scores(g, h):
            s = S[g]
            fb, po = h // 2, 64 * (h % 2)
            ets = [None] * JT[g]
            for grp in groups[g]:
                sct = sc.tile([128, 512], f32, tag="sc", name="sct")
                for i, jb in enumerate(grp):
                    jcol = O[g] + jb * 128
                    nc.tensor.matmul(
                        sct[:, i * s:(i + 1) * s],
                        kT_sb[fb][po:po + 64, jcol:jcol + 128],
                        qT_sb[fb][po:po + 64, P[g]:P[g] + s],
                        start=True, stop=True,
                        tile_position=(po, 0),
                    )
                et = epool.tile([128, 512], b16, tag="et", name="et")
                w = len(grp) * s
                nc.scalar.activation(et[:, :w], sct[:, :w], AF.Exp,
                                     scale=float(SCALE))
                for i, jb in enumerate(grp):
                    ets[jb] = et[:, i * s:(i + 1) * s]
            return ets

        def emit_attnv(g, h, ets):
            s = S[g]
            op = avp.tile([65, 512], f32, tag="av", name="avp")
            for jb in range(JT[g]):
                nc.tensor.matmul(
                    op[:, :s],
                    v_sb[(g, jb)][:, 65 * h:65 * h + 65],
                    ets[jb],
                    start=(jb == 0), stop=(jb == JT[g] - 1),
                )
            rc = rcpool.tile([1, 512], b16, tag="rc", name="rc")
            with nc.allow_low_precision("softmax 1/denom in bf16"):
                nc.vector.reciprocal(rc[:, :s], op[DH:DH + 1, :s])
            return op, rc

        def emit_norm(g, pair, op_even, rc_even, op_odd, rc_odd):
            s = S[g]
            bc = sc.tile([128, 512], f32, tag="sc", name="bc")
            nc.tensor.matmul(bc[0:64, :s], ones1[:, :], rc_even[:, :s],
                             start=True, stop=True, tile_position=(0, 0))
            nc.tensor.matmul(bc[64:128, :s], ones1[:, :], rc_odd[:, :s],
                             start=True, stop=True, tile_position=(0, 64))
            # GPSIMD cannot access PSUM: rb via ACT early (ACT has slack
            # between exps), via DVE for the late small-slot graphs where
            # the chain is latency-bound and DVE has idle gaps
            rb = rbpool.tile([128, 512], b16, tag="rb", name="rb")
            if g < NS - 2:
                nc.scalar.activation(rb[:, :s], bc[:, :s], AF.Copy)
            else:
                nc.vector.tensor_copy(rb[:, :s], bc[:, :s])
            fb = pair
            nc.vector.tensor_mul(cT_sb[fb][0:64, P[g]:P[g] + s],
                                 op_even[0:DH, :s], rb[0:64, :s])
            nc.vector.tensor_mul(cT_sb[fb][64:128, P[g]:P[g] + s],
                                 op_odd[0:DH, :s], rb[64:128, :s])

        # ---- global schedule: one continuous (g, h) step stream.
        # fill holds (graph_id, chunk); proj(g) is force-drained before
        # ATT(g) starts; otherwise drained adaptively so PE fill work is
        # spread over the whole attention stream.
        fill = deque()

        def force_proj(g):
            while any(gid == g for gid, _ in fill):
                fill.popleft()[1]()

        def drain_adaptive(iters_left):
            k = -(-len(fill) // max(1, iters_left))
            for _ in range(min(k, len(fill))):
                fill.popleft()[1]()

        p0 = (proj_chunks(0, sc, "sc"), proj_chunks(0))
        for i in range(len(p0[0])):
            p0[i % 2][i]()
        for g in range(1, NS):
            fill.extend((g, ch) for ch in proj_chunks(g))

        D_AV = 4
        ITERS = NS * H + D_AV  # one step per (g, h) plus pipeline tail
        pend = {}
        steps = [(g, h) for g in range(NS) for h in range(H)]
        for t in range(ITERS):
            tt = t - D_AV
            if 0 <= tt < len(steps):
                g, h = steps[tt]
                op, rc = emit_attnv(g, h, pend.pop((g, h)))
                pend[("op", g, h)] = (op, rc)
                if h % 2 == 1:
                    ope, rce = pend.pop(("op", g, h - 1))
                    opo, rco = pend.pop(("op", g, h))
                    emit_norm(g, h // 2, ope, rce, opo, rco)
                if h == H - 1:
                    fill.extend((NS, o_chunk(g, fb)) for fb in range(4))
            if t < len(steps):
                g, h = steps[t]
                if h == 0:
                    force_proj(g)
                pend[(g, h)] = emit_scores(g, h)
            drain_adaptive(ITERS - t)
        while fill:
            fill.popleft()[1]()

    _split_multiwaits(nc, mybir)
    return nc, dict(S=S, JT=JT, O=O, P=P, NPX=NPX, NPAD=NPAD, jbcol=jbcol)


def _split_multiwaits(nc, mybir, max_waits=1):
    """The pinned walrus codegen accepts only one sync-wait per instruction;
    move extra waits onto dedicated NoOps just before the instruction (same
    engine stream, so semantics are identical)."""
    n_split = 0
    for fn in nc.m.functions:
        for blk in fn.blocks:
            new_insts = []
            for inst in blk.instructions:
                si = getattr(inst, "sync_info", None)
                if si is not None and si.on_wait and len(si.on_wait) > max_waits:
                    waits = list(si.on_wait)
                    extra, keep = waits[:-max_waits], waits[-max_waits:]
                    for i, w in enumerate(extra):
                        new_insts.append(mybir.InstNoOp(
                            name=f"{inst.name}-w{i}",
                            sync_info=mybir.SyncInfo(on_wait=[w], on_update=[]),
                            engine=inst.engine,
                            bass_nofuse=True,
                        ))
                    inst.sync_info = mybir.SyncInfo(on_wait=keep,
                                                    on_update=si.on_update)
                    n_split += 1
                new_insts.append(inst)
            blk.instructions = new_insts
    return n_split


def _get_nc(S):
    key = tuple(S)
    if key not in _NC_CACHE:
        _NC_CACHE[key] = _build(key)
    return _NC_CACHE[key]


def _plan(counts):
    order = np.argsort(-counts, kind="stable")
    S = []
    for k in range(NS):
        band = counts[order[k * N_CORES:(k + 1) * N_CORES]]
        S.append(max(8, -(-int(band.max()) // 4) * 4))
    return order, tuple(S)


def kernel(x, batch, w_q, w_k, w_v, b_q, b_k, b_v, w_o, b_o):
    global LAST_RESULTS
    x = np.asarray(x, np.float32)
    batch = np.asarray(batch)
    counts = np.bincount(batch, minlength=NUM_GRAPHS)[:NUM_GRAPHS]
    starts = np.concatenate([[0], np.cumsum(counts)]).astype(np.int64)
    order, S = _plan(counts)
    assert max(S) <= 512, f"graph too large: {counts.max()}"
    nc, meta = _get_nc(S)
    JT, O, P = meta["JT"], meta["O"], meta["P"]
    NPX, NPAD, jbcol = meta["NPX"], meta["NPAD"], meta["jbcol"]
    TOTJB = len(jbcol)

    bf16 = ml_dtypes.bfloat16
    fp8 = ml_dtypes.float8_e4m3

    def pair_hilo(mT):
        # mT [512, cols] fp32 (pre-scaled) -> {('h'|'l', t): [128, 2, cols]}
        hi = mT.astype(fp8)
        lo = (mT - hi.astype(np.float32)).astype(fp8)
        out = {}
        for hl, m in (("h", hi), ("l", lo)):
            for t in range(2):
                out[(hl, t)] = np.ascontiguousarray(
                    m[256 * t:256 * (t + 1)].reshape(2, 128, -1)
                    .transpose(1, 0, 2))
        return out

    woT = np.ascontiguousarray(w_o.T).astype(bf16)
    wq8 = pair_hilo(np.ascontiguousarray(w_q.T) * SW)
    wk8 = pair_hilo(np.ascontiguousarray(w_k.T) * SW)
    wvT = np.zeros((DIM, VC), np.float32)
    for h in range(H):
        wvT[:, 65 * h:65 * h + 64] = w_v[64 * h:64 * h + 64, :].T
    wv8 = pair_hilo(wvT * SW)
    bq = np.ascontiguousarray(b_q.reshape(4, 128).T.astype(np.float32))
    bo2v = (b_o + w_o @ b_v).astype(np.float32)
    bo2 = np.ascontiguousarray(bo2v.reshape(4, 128).T)

    in_maps = []
    for c in range(N_CORES):
        xs = np.zeros((NPX, DIM), np.float32)
        vmask = np.zeros((128, TOTJB), np.float32)
        for k in range(NS):
            g = order[k * N_CORES + c]
            n = int(counts[g])
            xs[O[k]:O[k] + n] = x[starts[g]:starts[g] + n]
            for jb in range(JT[k]):
                nvalid = min(128, max(0, n - jb * 128))
                vmask[:nvalid, jbcol[(k, jb)]] = 1.0
        vones = np.repeat(vmask, 8, axis=1)
        x8 = pair_hilo(np.ascontiguousarray(xs.T) * SX)
        im = {"woT": woT, "bq": bq, "bo2": bo2,
              "vones": np.ascontiguousarray(vones).astype(bf16)}
        for (hl, t), v8 in x8.items():
            im[f"x{hl}{t}"] = v8
        for w, w8 in (("q", wq8), ("k", wk8), ("v", wv8)):
            for (hl, t), v8 in w8.items():
                im[f"w{w}{hl}{t}"] = v8
        in_maps.append(im)

    from concourse.bass_utils import run_bass_kernel_spmd
    trace = os.environ.get("KTRACE", "") not in ("", "0")
    LAST_RESULTS = run_bass_kernel_spmd(nc, in_maps, list(range(N_CORES)),
                                        trace=trace)

    out = np.empty((x.shape[0], DIM), np.float32)
    for c in range(N_CORES):
        oT = LAST_RESULTS.results[c]["outT"]
        for k in range(NS):
            g = order[k * N_CORES + c]
            n = int(counts[g])
            out[starts[g]:starts[g] + n] = oT[:, P[k]:P[k] + n].T.astype(np.float32)
    return out
